# revision 40
# baseline (speedup 1.0000x reference)
"""Expert-parallel MoE SwiGLU kernel for 8 Trainium2 NeuronCores.

Strategy: expert parallelism with host-side dispatch/combine. Each of the
8 cores owns one expert's weights; the host routes tokens by expert_idx and
packs each expert's tokens as a transposed [128, ND*W] slab (features on
partitions, no on-chip transposes). Each core runs a dense SwiGLU FFN.

Kernel structure: a flat software pipeline over the 32 f-chunks of d_ff.
Per chunk: one fused 512KB gate+up weight DMA, 8+8 gate/up matmuls
(fp16 operands, fp32 PSUM), silu on ACT, t = silu(g)*u on DVE (fp16).
The down-projection accumulates in PSUM over uneven f-chunk groups
[2,6,8,8,7,1]; group g's down matmuls are interleaved into group g+1's
slots so the PE never waits, and the tiny last group keeps the tail
short: its 8 accumulators borrow the idle gate/up PSUM banks, the
accumulated y folds in via an identity matmul on the PE, and the
PSUM->SBUF copies alternate DVE/ACT before per-dt output DMAs on both
queues. Ramp: the two HWDGE rings fair-share HBM bandwidth, so
transfers are ordered in consumption order across rings at >=256KB
granularity (x halves head both rings, chunk 0/1 gate halves first, wd
deferred); dummy matmuls warm the PE activity monitor at t=0 and slice
ramp DMA-wait gaps below the HAM idle window so real matmuls run at
2.4GHz throughout.

Relative error vs the fp32 reference: ~1e-3 (fp16 operands/output,
fp32 PSUM accumulation).
"""

import numpy as np
from contextlib import ExitStack

D_MODEL = 1024
D_FF = 4096
N_EXPERTS = 8
N_CORES = 8

_ND = D_MODEL // 128   # 8 d-chunks
_NF = D_FF // 128      # 32 f-chunks
_GS = [2, 6, 8, 8, 7, 1]   # down-proj PSUM accumulation group sizes

_PREFETCH_WGU = 6

_nc_cache = {}


def _groups():
    out, s = [], 0
    for n in _GS:
        out.append(list(range(s, s + n)))
        s += n
    return out


def _build_nc(W: int):
    """Build + schedule the per-core Bass program for token capacity W."""
    import concourse.bacc as bacc
    import concourse.tile as tile
    from concourse import mybir

    f32 = mybir.dt.float32
    f16 = mybir.dt.float16

    nc = bacc.Bacc("TRN2", target_bir_lowering=False, debug=False,
                   num_devices=N_CORES)
    xt = nc.dram_tensor("xt", [128, _ND * W], f16, kind="ExternalInput").ap()
    wgu = nc.dram_tensor("wgu", [_NF, 128, 2 * _ND * 128], f16,
                         kind="ExternalInput").ap()
    wd = nc.dram_tensor("wd", [_NF, 128, D_MODEL], f16,
                        kind="ExternalInput").ap()
    ident = nc.dram_tensor("ident", [128, 128], f16,
                           kind="ExternalInput").ap()
    yt = nc.dram_tensor("yt", [128, _ND * W], f16, kind="ExternalOutput").ap()

    groups = _groups()
    G = len(groups)

    with tile.TileContext(nc) as tc, ExitStack() as ctx:
        xp = ctx.enter_context(tc.tile_pool(name="xp", bufs=1))
        wgup = ctx.enter_context(tc.tile_pool(name="wgup", bufs=_PREFETCH_WGU))
        wdp = ctx.enter_context(tc.tile_pool(name="wdp", bufs=20))
        tp = ctx.enter_context(tc.tile_pool(name="tp", bufs=18))
        gap = ctx.enter_context(tc.tile_pool(name="gap", bufs=3))
        yp = ctx.enter_context(tc.tile_pool(name="yp", bufs=1))
        pg = ctx.enter_context(tc.tile_pool(name="pg", bufs=2, space="PSUM"))
        pu = ctx.enter_context(tc.tile_pool(name="pu", bufs=2, space="PSUM"))
        pd = ctx.enter_context(tc.tile_pool(name="pd", bufs=4, space="PSUM"))

        y_t = yp.tile([128, _ND * W], f16, tag="y", name="y_t")
        id_t = yp.tile([128, 128], f16, tag="id", name="id_t")

        # HAM warm-up scratch (memset on DVE at t=0, dummies on PE fill the
        # cold-start idle slices so real matmuls run warm at 2.4GHz).
        scr_w = xp.tile([128, 128], f16, tag="scrw", name="scr_w")
        scr_x = xp.tile([128, W], f16, tag="scrx", name="scr_x")
        nc.vector.memset(scr_w[:], 0.0)
        nc.vector.memset(scr_x[:], 0.0)
        _scr_n = [0]

        def emit_warmup(n):
            for _ in range(n):
                i = _scr_n[0]
                _scr_n[0] += 1
                p = pd.tile([128, W], f32, tag="pd", name=f"scr_p{i}")
                nc.tensor.matmul(p[:], scr_w[:], scr_x[:],
                                 start=True, stop=True)

        wgu_tiles = {}
        wd_tiles = {}
        t_tiles = {}
        pd_tiles = {}

        # Ramp: the two HWDGE rings fair-share HBM bandwidth, so order
        # transfers in consumption order across rings with efficient
        # (>=256KB) sizes: chunk 0/1 gate halves lead, x halves head both
        # rings, wd transfers defer behind the first wgu chunks.
        x_lo = xp.tile([128, 4 * W], f16, tag="xlo", name="x_lo")
        x_hi = xp.tile([128, 4 * W], f16, tag="xhi", name="x_hi")

        def x_slice(d):
            if d < 4:
                return x_lo[:, d * W:(d + 1) * W]
            return x_hi[:, (d - 4) * W:(d - 3) * W]

        def issue_wgu(ft, split=False):
            t = wgup.tile([128, 2 * _ND * 128], f16, tag="wgu",
                          name=f"wgu{ft}")
            eng = nc.sync if ft % 2 == 0 else nc.scalar
            if split:
                # gate half first: chunk's gate matmuls start on 256KB
                eng.dma_start(t[:, :_ND * 128], wgu[ft][:, :_ND * 128])
                eng.dma_start(t[:, _ND * 128:], wgu[ft][:, _ND * 128:])
            else:
                eng.dma_start(t[:], wgu[ft])
            wgu_tiles[ft] = t

        def issue_wd(ft):
            t = wdp.tile([128, D_MODEL], f16, tag="wd", name=f"wd{ft}")
            eng = nc.scalar if ft % 2 == 0 else nc.sync
            eng.dma_start(t[:], wd[ft])
            wd_tiles[ft] = t

        # pre-roll in consumption order
        nc.scalar.dma_start(x_lo[:], xt[:, :4 * W])
        t0 = wgup.tile([128, 2 * _ND * 128], f16, tag="wgu", name="wgu0")
        wgu_tiles[0] = t0
        nc.sync.dma_start(t0[:, :_ND * 128], wgu[0][:, :_ND * 128])
        nc.sync.dma_start(x_hi[:], xt[:, 4 * W:])
        nc.sync.dma_start(t0[:, _ND * 128:], wgu[0][:, _ND * 128:])
        issue_wgu(1, split=True)
        issue_wgu(2)
        issue_wgu(3)
        issue_wd(0)
        issue_wd(1)
        issue_wgu(4)
        issue_wgu(5)
        issue_wd(2)
        nc.scalar.dma_start(id_t[:], ident[:, :])

        def emit_chunk(c):
            psg = pg.tile([128, W], f32, tag="pg", name=f"psg{c}")
            for d in range(_ND):
                nc.tensor.matmul(psg[:],
                                 wgu_tiles[c][:, d * 128:(d + 1) * 128],
                                 x_slice(d),
                                 start=(d == 0), stop=(d == _ND - 1))
                if c == 0 and d == 3:
                    emit_warmup(2)
            # silu emitted before the up matmuls: ACT overlaps them, so the
            # t tile is ready one DVE-mul after the up accumulation stops
            ga = gap.tile([128, W], f32, tag="ga", name=f"ga{c}")
            nc.scalar.activation(ga[:], psg[:],
                                 mybir.ActivationFunctionType.Silu)
            psu = pu.tile([128, W], f32, tag="pu", name=f"psu{c}")
            for d in range(_ND):
                nc.tensor.matmul(psu[:],
                                 wgu_tiles[c][:, (_ND + d) * 128:
                                              (_ND + d + 1) * 128],
                                 x_slice(d),
                                 start=(d == 0), stop=(d == _ND - 1))
            tt = tp.tile([128, W], f16, tag="t", name=f"t{c}")
            nc.vector.tensor_mul(tt[:], ga[:], psu[:])
            t_tiles[c] = tt

        def down_items(g):
            # (dt, j, first, last) in dt-major order: pd bank per dt
            # accumulates the group's f-chunks, then one DVE add into y.
            chs = groups[g]
            return [(dt, j, j == 0, j == len(chs) - 1)
                    for dt in range(_ND)
                    for j in range(len(chs))]

        def emit_down_item(g, item, tail=False):
            dt, j, first, last = item
            ftl = groups[g][j]
            if first:
                if tail:
                    # borrow the idle gate/up PSUM banks so the 8 tail
                    # accumulators never stall on the 4-deep pd ring
                    pool, ptag = ((pd, "pd"), (pd, "pd"), (pd, "pd"),
                                  (pd, "pd"), (pg, "pg"), (pg, "pg"),
                                  (pu, "pu"), (pu, "pu"))[dt]
                else:
                    pool, ptag = pd, "pd"
                pd_tiles[dt] = pool.tile([128, W], f32, tag=ptag,
                                         name=f"pd{g}_{dt}")
            pdt = pd_tiles[dt]
            nc.tensor.matmul(pdt[:],
                             wd_tiles[ftl][:, dt * 128:(dt + 1) * 128],
                             t_tiles[ftl][:], start=first,
                             stop=(last and not tail))
            if last:
                ys = y_t[:, dt * W:(dt + 1) * W]
                if tail:
                    # fold the accumulated y into PSUM with an identity
                    # matmul (PE), then split the PSUM->SBUF copies between
                    # DVE and ACT so the 8 tail merges don't serialize on
                    # one engine; output ships as two half-width DMAs (one
                    # per queue) to amortize the ~0.6us per-DMA issue cost.
                    nc.tensor.matmul(pdt[:], id_t[:], ys,
                                     start=False, stop=True)
                    if dt % 2 == 0:
                        nc.vector.tensor_copy(ys, pdt[:])
                    else:
                        nc.scalar.copy(ys, pdt[:])
                    if dt == 3:
                        nc.sync.dma_start(yt[:, :4 * W], y_t[:, :4 * W])
                    elif dt == 7:
                        nc.scalar.dma_start(yt[:, 4 * W:], y_t[:, 4 * W:])
                elif g == 0:
                    nc.vector.tensor_copy(ys, pdt[:])
                else:
                    nc.vector.tensor_add(ys, ys, pdt[:])

        emit_warmup(6)

        pending = []   # down items of the previous group
        for g, chs in enumerate(groups):
            n_slots = len(chs)
            total = len(pending)
            done = 0
            for si, c in enumerate(chs):
                if c + _PREFETCH_WGU < _NF:
                    issue_wgu(c + _PREFETCH_WGU)
                if c + 3 < _NF:
                    issue_wd(c + 3)
                # previous group's downs lead the slot: their operands are
                # already on-chip, so a late-arriving weight chunk stalls
                # nothing — the PE chews downs while the DMA catches up
                quota = (total * (si + 1)) // n_slots
                while done < quota:
                    emit_down_item(g - 1, pending[done])
                    done += 1
                emit_chunk(c)
                if 1 <= c < 10:
                    # slice any ramp DMA-wait gaps below the ~3.4us HAM
                    # idle window (few down matmuls exist yet to fill the
                    # slots): the PE stays at 2.4GHz through the weight
                    # stream's catch-up instead of re-throttling
                    emit_warmup(1)
            pending = down_items(g)

        # tail: the final (single-chunk) group's downs, adds and y DMAs
        for item in pending:
            emit_down_item(G - 1, item, tail=True)

    nc.compile()
    return nc


def _pack_weights(w_gate, w_up, w_down):
    """Per-expert packed fp16 weight blobs matching the kernel layout."""
    packed = []
    for e in range(N_EXPERTS):
        wg = np.asarray(w_gate[e], dtype=np.float32).astype(np.float16)
        wu = np.asarray(w_up[e], dtype=np.float32).astype(np.float16)
        # [D, F] -> [ft, p, d, c]
        g4 = wg.reshape(_ND, 128, _NF, 128).transpose(2, 1, 0, 3)
        u4 = wu.reshape(_ND, 128, _NF, 128).transpose(2, 1, 0, 3)
        wgu = np.ascontiguousarray(
            np.stack([g4, u4], axis=2).reshape(_NF, 128, 2 * _ND * 128))
        wdp = np.ascontiguousarray(
            np.asarray(w_down[e], dtype=np.float32).astype(np.float16)
            .reshape(_NF, 128, D_MODEL))
        packed.append((wgu, wdp))
    return packed


def _pack_x(x_flat, toks, W):
    """[T, D] rows toks -> [128, ND*W] fp16 slab (d-chunk on partitions)."""
    xe = np.zeros((128, _ND * W), dtype=np.float16)
    n = len(toks)
    if n:
        # [n, D] -> [D, n] -> [ND, 128, n] -> [128, ND, n]
        a = x_flat[toks].T.astype(np.float16).reshape(_ND, 128, n)
        xe.reshape(128, _ND, W)[:, :, :n] = a.transpose(1, 0, 2)
    return xe


_IDENT = np.eye(128, dtype=np.float16)


def _make_in_maps(x_flat, tok_lists, packed_w, W):
    in_maps = []
    for e in range(N_EXPERTS):
        wgu, wdp = packed_w[e]
        in_maps.append({
            "xt": _pack_x(x_flat, tok_lists[e], W),
            "wgu": wgu,
            "wd": wdp,
            "ident": _IDENT,
        })
    return in_maps


def _run_one(W, tok_lists, x_flat, packed_w, out_flat):
    from concourse.bass_utils import run_bass_kernel_spmd

    if W not in _nc_cache:
        _nc_cache[W] = _build_nc(W)
    nc = _nc_cache[W]

    in_maps = _make_in_maps(x_flat, tok_lists, packed_w, W)

    res = None
    for attempt in range(3):
        try:
            res = run_bass_kernel_spmd(nc, in_maps,
                                       core_ids=list(range(N_CORES)))
            break
        except Exception:
            if attempt == 2:
                raise
            import time
            time.sleep(3.0)
            try:
                import jax
                jax.clear_caches()
                jax.clear_backends()
            except Exception:
                pass
    for e in range(N_EXPERTS):
        toks = tok_lists[e]
        n = len(toks)
        if n:
            y = res.results[e]["yt"].reshape(128, _ND, W)[:, :, :n]
            out_flat[toks] = y.transpose(1, 0, 2).reshape(D_MODEL, n).T


def kernel(x, expert_idx, w_gate, w_up, w_down):
    x = np.asarray(x, dtype=np.float32)
    idx = np.asarray(expert_idx).astype(np.int64)
    B, S, D = x.shape
    T = B * S
    x_flat = np.ascontiguousarray(x.reshape(T, D))
    idx_flat = idx.reshape(T)

    packed_w = _pack_weights(w_gate, w_up, w_down)

    tok_lists = [np.nonzero(idx_flat == e)[0] for e in range(N_EXPERTS)]
    cap = max(1, max(len(t) for t in tok_lists))
    out_flat = np.zeros((T, D), dtype=np.float32)

    if cap <= 512:
        # normal path: one SPMD run, capacity = max expert load (floor 128
        # keeps tile shapes sane for tiny inputs)
        W = max(128, cap)
        _run_one(W, tok_lists, x_flat, packed_w, out_flat)
    else:
        # fallback for extreme routing imbalance: process tokens in
        # rounds of <=512 per expert, reusing one compiled W=512 program
        rounds = -(-cap // 512)
        for r in range(rounds):
            round_lists = [t[r * 512:(r + 1) * 512] for t in tok_lists]
            _run_one(512, round_lists, x_flat, packed_w, out_flat)

    return out_flat.reshape(B, S, D)


# revision 42
# speedup vs baseline: 1.0206x; 1.0206x over previous
"""Expert-parallel MoE SwiGLU kernel for 8 Trainium2 NeuronCores.

Strategy: expert parallelism with host-side dispatch/combine. Each of the
8 cores owns one expert's weights; the host routes tokens by expert_idx and
packs each expert's tokens as a transposed [128, ND*W] slab (features on
partitions, no on-chip transposes). Each core runs a dense SwiGLU FFN.

Kernel structure: a flat software pipeline over the 32 f-chunks of d_ff.
Per chunk: one fused 512KB gate+up weight DMA, 8+8 gate/up matmuls
(fp16 operands, fp32 PSUM), silu on ACT, t = silu(g)*u on DVE (fp16).
The down-projection accumulates in PSUM over uneven f-chunk groups
[2,6,8,8,7,1]; group g's down matmuls are interleaved into group g+1's
slots so the PE never waits, and the tiny last group keeps the tail
short: its 8 accumulators borrow the idle gate/up PSUM banks, the
accumulated y folds in via an identity matmul on the PE, and the
PSUM->SBUF copies alternate DVE/ACT before per-dt output DMAs on both
queues. Ramp: the two HWDGE rings fair-share HBM bandwidth, so
transfers are ordered in consumption order across rings at >=256KB
granularity (x halves head both rings, chunk 0/1 gate halves first, wd
deferred); dummy matmuls warm the PE activity monitor at t=0 and slice
ramp DMA-wait gaps below the HAM idle window so real matmuls run at
2.4GHz throughout.

Relative error vs the fp32 reference: ~1e-3 (fp16 operands/output,
fp32 PSUM accumulation).
"""

import numpy as np
from contextlib import ExitStack

D_MODEL = 1024
D_FF = 4096
N_EXPERTS = 8
N_CORES = 8

_ND = D_MODEL // 128   # 8 d-chunks
_NF = D_FF // 128      # 32 f-chunks
_GS = [2, 6, 8, 8, 7, 1]   # down-proj PSUM accumulation group sizes

_PREFETCH_WGU = 6

_nc_cache = {}


def _groups():
    out, s = [], 0
    for n in _GS:
        out.append(list(range(s, s + n)))
        s += n
    return out


def _build_nc(W: int):
    """Build + schedule the per-core Bass program for token capacity W."""
    import concourse.bacc as bacc
    import concourse.tile as tile
    from concourse import mybir

    f32 = mybir.dt.float32
    f16 = mybir.dt.float16

    nc = bacc.Bacc("TRN2", target_bir_lowering=False, debug=False,
                   num_devices=N_CORES)
    xt = nc.dram_tensor("xt", [128, _ND * W], f16, kind="ExternalInput").ap()
    wgu = nc.dram_tensor("wgu", [_NF, 128, 2 * _ND * 128], f16,
                         kind="ExternalInput").ap()
    wd = nc.dram_tensor("wd", [_NF, 128, D_MODEL], f16,
                        kind="ExternalInput").ap()
    ident = nc.dram_tensor("ident", [128, 128], f16,
                           kind="ExternalInput").ap()
    yt = nc.dram_tensor("yt", [128, _ND * W], f16, kind="ExternalOutput").ap()

    groups = _groups()
    G = len(groups)

    with tile.TileContext(nc) as tc, ExitStack() as ctx:
        xp = ctx.enter_context(tc.tile_pool(name="xp", bufs=1))
        wgup = ctx.enter_context(tc.tile_pool(name="wgup", bufs=_PREFETCH_WGU))
        wdp = ctx.enter_context(tc.tile_pool(name="wdp", bufs=20))
        tp = ctx.enter_context(tc.tile_pool(name="tp", bufs=18))
        gap = ctx.enter_context(tc.tile_pool(name="gap", bufs=3))
        yp = ctx.enter_context(tc.tile_pool(name="yp", bufs=1))
        pg = ctx.enter_context(tc.tile_pool(name="pg", bufs=2, space="PSUM"))
        pu = ctx.enter_context(tc.tile_pool(name="pu", bufs=2, space="PSUM"))
        pd = ctx.enter_context(tc.tile_pool(name="pd", bufs=4, space="PSUM"))

        y_t = yp.tile([128, _ND * W], f16, tag="y", name="y_t")
        id_t = yp.tile([128, 128], f16, tag="id", name="id_t")

        # HAM warm-up scratch (memset on DVE at t=0, dummies on PE fill the
        # cold-start idle slices so real matmuls run warm at 2.4GHz).
        scr_w = xp.tile([128, 128], f16, tag="scrw", name="scr_w")
        scr_x = xp.tile([128, W], f16, tag="scrx", name="scr_x")
        nc.vector.memset(scr_w[:], 0.0)
        nc.vector.memset(scr_x[:], 0.0)
        _scr_n = [0]

        def emit_warmup(n):
            for _ in range(n):
                i = _scr_n[0]
                _scr_n[0] += 1
                p = pd.tile([128, W], f32, tag="pd", name=f"scr_p{i}")
                nc.tensor.matmul(p[:], scr_w[:], scr_x[:],
                                 start=True, stop=True)

        wgu_tiles = {}
        wd_tiles = {}
        t_tiles = {}
        pd_tiles = {}

        # Ramp: the two HWDGE rings fair-share HBM bandwidth, so order
        # transfers in consumption order across rings with efficient
        # (>=256KB) sizes: chunk 0/1 gate halves lead, x halves head both
        # rings, wd transfers defer behind the first wgu chunks.
        x_lo = xp.tile([128, 4 * W], f16, tag="xlo", name="x_lo")
        x_hi = xp.tile([128, 4 * W], f16, tag="xhi", name="x_hi")

        def x_slice(d):
            if d < 4:
                return x_lo[:, d * W:(d + 1) * W]
            return x_hi[:, (d - 4) * W:(d - 3) * W]

        def issue_wgu(ft, split=False):
            t = wgup.tile([128, 2 * _ND * 128], f16, tag="wgu",
                          name=f"wgu{ft}")
            eng = nc.sync if ft % 2 == 0 else nc.scalar
            if split:
                # gate half first: chunk's gate matmuls start on 256KB
                eng.dma_start(t[:, :_ND * 128], wgu[ft][:, :_ND * 128])
                eng.dma_start(t[:, _ND * 128:], wgu[ft][:, _ND * 128:])
            else:
                eng.dma_start(t[:], wgu[ft])
            wgu_tiles[ft] = t

        def issue_wd(ft):
            t = wdp.tile([128, D_MODEL], f16, tag="wd", name=f"wd{ft}")
            eng = nc.scalar if ft % 2 == 0 else nc.sync
            eng.dma_start(t[:], wd[ft])
            wd_tiles[ft] = t

        # pre-roll in consumption order
        nc.scalar.dma_start(x_lo[:], xt[:, :4 * W])
        t0 = wgup.tile([128, 2 * _ND * 128], f16, tag="wgu", name="wgu0")
        wgu_tiles[0] = t0
        nc.sync.dma_start(t0[:, :_ND * 128], wgu[0][:, :_ND * 128])
        nc.sync.dma_start(x_hi[:], xt[:, 4 * W:])
        nc.sync.dma_start(t0[:, _ND * 128:], wgu[0][:, _ND * 128:])
        issue_wgu(1, split=True)
        issue_wgu(2)
        issue_wgu(3)
        issue_wd(0)
        issue_wd(1)
        issue_wgu(4)
        issue_wgu(5)
        issue_wd(2)
        nc.scalar.dma_start(id_t[:], ident[:, :])

        def emit_chunk(c):
            psg = pg.tile([128, W], f32, tag="pg", name=f"psg{c}")
            for d in range(_ND):
                nc.tensor.matmul(psg[:],
                                 wgu_tiles[c][:, d * 128:(d + 1) * 128],
                                 x_slice(d),
                                 start=(d == 0), stop=(d == _ND - 1))
                if c == 0 and d == 3:
                    emit_warmup(2)
            psu = pu.tile([128, W], f32, tag="pu", name=f"psu{c}")
            for d in range(_ND):
                nc.tensor.matmul(psu[:],
                                 wgu_tiles[c][:, (_ND + d) * 128:
                                              (_ND + d + 1) * 128],
                                 x_slice(d),
                                 start=(d == 0), stop=(d == _ND - 1))
            ga = gap.tile([128, W], f32, tag="ga", name=f"ga{c}")
            nc.scalar.activation(ga[:], psg[:],
                                 mybir.ActivationFunctionType.Silu)
            tt = tp.tile([128, W], f16, tag="t", name=f"t{c}")
            nc.vector.tensor_mul(tt[:], ga[:], psu[:])
            t_tiles[c] = tt

        def down_items(g):
            # (dt, j, first, last) in dt-major order: pd bank per dt
            # accumulates the group's f-chunks, then one DVE add into y.
            chs = groups[g]
            return [(dt, j, j == 0, j == len(chs) - 1)
                    for dt in range(_ND)
                    for j in range(len(chs))]

        def emit_down_item(g, item, tail=False):
            dt, j, first, last = item
            ftl = groups[g][j]
            if first:
                if tail:
                    # borrow the idle gate/up PSUM banks so the 8 tail
                    # accumulators never stall on the 4-deep pd ring
                    pool, ptag = ((pd, "pd"), (pd, "pd"), (pd, "pd"),
                                  (pd, "pd"), (pg, "pg"), (pg, "pg"),
                                  (pu, "pu"), (pu, "pu"))[dt]
                else:
                    pool, ptag = pd, "pd"
                pd_tiles[dt] = pool.tile([128, W], f32, tag=ptag,
                                         name=f"pd{g}_{dt}")
            pdt = pd_tiles[dt]
            nc.tensor.matmul(pdt[:],
                             wd_tiles[ftl][:, dt * 128:(dt + 1) * 128],
                             t_tiles[ftl][:], start=first,
                             stop=(last and not tail))
            if last:
                ys = y_t[:, dt * W:(dt + 1) * W]
                if tail:
                    # fold the accumulated y into PSUM with an identity
                    # matmul (PE), then split the PSUM->SBUF copies between
                    # DVE and ACT so the 8 tail merges don't serialize on
                    # one engine; output ships as two half-width DMAs (one
                    # per queue) to amortize the ~0.6us per-DMA issue cost.
                    nc.tensor.matmul(pdt[:], id_t[:], ys,
                                     start=False, stop=True)
                    if dt % 2 == 0:
                        nc.vector.tensor_copy(ys, pdt[:])
                    else:
                        nc.scalar.copy(ys, pdt[:])
                    if dt == 3:
                        nc.sync.dma_start(yt[:, :4 * W], y_t[:, :4 * W])
                    elif dt == 7:
                        nc.scalar.dma_start(yt[:, 4 * W:], y_t[:, 4 * W:])
                elif g == 0:
                    nc.vector.tensor_copy(ys, pdt[:])
                else:
                    nc.vector.tensor_add(ys, ys, pdt[:])

        emit_warmup(6)

        pending = []   # down items of the previous group
        for g, chs in enumerate(groups):
            n_slots = len(chs)
            total = len(pending)
            done = 0
            for si, c in enumerate(chs):
                if c + _PREFETCH_WGU < _NF:
                    issue_wgu(c + _PREFETCH_WGU)
                if c + 3 < _NF:
                    issue_wd(c + 3)
                emit_chunk(c)
                if 1 <= c < 10:
                    # slice any ramp DMA-wait gaps below the ~3.4us HAM
                    # idle window (few down matmuls exist yet to fill the
                    # slots): the PE stays at 2.4GHz through the weight
                    # stream's catch-up instead of re-throttling
                    emit_warmup(1)
                quota = (total * (si + 1)) // n_slots
                while done < quota:
                    emit_down_item(g - 1, pending[done])
                    done += 1
            pending = down_items(g)

        # tail: the final (single-chunk) group's downs, adds and y DMAs
        for item in pending:
            emit_down_item(G - 1, item, tail=True)

    nc.compile()
    return nc


def _pack_weights(w_gate, w_up, w_down):
    """Per-expert packed fp16 weight blobs matching the kernel layout."""
    packed = []
    for e in range(N_EXPERTS):
        wg = np.asarray(w_gate[e], dtype=np.float32).astype(np.float16)
        wu = np.asarray(w_up[e], dtype=np.float32).astype(np.float16)
        # [D, F] -> [ft, p, d, c]
        g4 = wg.reshape(_ND, 128, _NF, 128).transpose(2, 1, 0, 3)
        u4 = wu.reshape(_ND, 128, _NF, 128).transpose(2, 1, 0, 3)
        wgu = np.ascontiguousarray(
            np.stack([g4, u4], axis=2).reshape(_NF, 128, 2 * _ND * 128))
        wdp = np.ascontiguousarray(
            np.asarray(w_down[e], dtype=np.float32).astype(np.float16)
            .reshape(_NF, 128, D_MODEL))
        packed.append((wgu, wdp))
    return packed


def _pack_x(x_flat, toks, W):
    """[T, D] rows toks -> [128, ND*W] fp16 slab (d-chunk on partitions)."""
    xe = np.zeros((128, _ND * W), dtype=np.float16)
    n = len(toks)
    if n:
        # [n, D] -> [D, n] -> [ND, 128, n] -> [128, ND, n]
        a = x_flat[toks].T.astype(np.float16).reshape(_ND, 128, n)
        xe.reshape(128, _ND, W)[:, :, :n] = a.transpose(1, 0, 2)
    return xe


_IDENT = np.eye(128, dtype=np.float16)


def _make_in_maps(x_flat, tok_lists, packed_w, W):
    in_maps = []
    for e in range(N_EXPERTS):
        wgu, wdp = packed_w[e]
        in_maps.append({
            "xt": _pack_x(x_flat, tok_lists[e], W),
            "wgu": wgu,
            "wd": wdp,
            "ident": _IDENT,
        })
    return in_maps


def _run_one(W, tok_lists, x_flat, packed_w, out_flat):
    from concourse.bass_utils import run_bass_kernel_spmd

    if W not in _nc_cache:
        _nc_cache[W] = _build_nc(W)
    nc = _nc_cache[W]

    in_maps = _make_in_maps(x_flat, tok_lists, packed_w, W)

    res = None
    for attempt in range(3):
        try:
            res = run_bass_kernel_spmd(nc, in_maps,
                                       core_ids=list(range(N_CORES)))
            break
        except Exception:
            if attempt == 2:
                raise
            import time
            time.sleep(3.0)
            try:
                import jax
                jax.clear_caches()
                jax.clear_backends()
            except Exception:
                pass
    for e in range(N_EXPERTS):
        toks = tok_lists[e]
        n = len(toks)
        if n:
            y = res.results[e]["yt"].reshape(128, _ND, W)[:, :, :n]
            out_flat[toks] = y.transpose(1, 0, 2).reshape(D_MODEL, n).T


def kernel(x, expert_idx, w_gate, w_up, w_down):
    x = np.asarray(x, dtype=np.float32)
    idx = np.asarray(expert_idx).astype(np.int64)
    B, S, D = x.shape
    T = B * S
    x_flat = np.ascontiguousarray(x.reshape(T, D))
    idx_flat = idx.reshape(T)

    packed_w = _pack_weights(w_gate, w_up, w_down)

    tok_lists = [np.nonzero(idx_flat == e)[0] for e in range(N_EXPERTS)]
    cap = max(1, max(len(t) for t in tok_lists))
    out_flat = np.zeros((T, D), dtype=np.float32)

    if cap <= 512:
        # normal path: one SPMD run, capacity = max expert load (floor 128
        # keeps tile shapes sane for tiny inputs)
        W = max(128, cap)
        _run_one(W, tok_lists, x_flat, packed_w, out_flat)
    else:
        # fallback for extreme routing imbalance: process tokens in
        # rounds of <=512 per expert, reusing one compiled W=512 program
        rounds = -(-cap // 512)
        for r in range(rounds):
            round_lists = [t[r * 512:(r + 1) * 512] for t in tok_lists]
            _run_one(512, round_lists, x_flat, packed_w, out_flat)

    return out_flat.reshape(B, S, D)
